# revision 3
# baseline (speedup 1.0000x reference)
"""Trainium2 Bass kernel for nn_CosineSimilarityLayer.

out = l2norm_rows(x) @ (l2norm_rows_over_N(W))            x:[4096,512] W:[512,5994]

Math:  out[b,n] = xscale[b] * sum_d x[b,d] * wscale[d] * W[d,n]
  xscale[b] = rsqrt(max(sum_d x[b,d]^2, eps))   (folded into PSUM eviction)
  wscale[d] = rsqrt(max(sum_n W[d,n]^2, eps))   (folded into transposed x)

Sharding: data-parallel over batch, 8 cores x [512, 512] shards, W replicated.
The W row-norm (a full-N reduction) is computed cooperatively: each core
squares+reduces a 750-col slice of W, then an 8-core AllReduce produces the
full sums, so matmuls start ~15us in instead of waiting for a full W pass.

Matmul runs in float32r (fast fp32 mode, ~13-bit mantissa per pass).  NPASS
controls precision: each operand is split into float32r value + float32r
residual; 3 passes (hi*hi + lo*hi + hi*lo) recovers ~fp32 accuracy.
"""

import os
import sys
import types
from contextlib import ExitStack

import numpy as np


def _ensure_axon_hooks():
    """bass_utils' trace path imports antenv.axon_hooks, which some images
    lack.  Provide it (wired to the ctypes NTFF hook when available) so
    BASS_TRACE=1 profiles instead of crashing.  No-op when already present."""
    try:
        import antenv.axon_hooks  # noqa: F401
        return
    except ImportError:
        pass
    try:
        import antenv
    except ImportError:
        return
    m = types.ModuleType("antenv.axon_hooks")
    holder = {"h": None}
    m.set_axon_ntff_profile_hook = lambda h: holder.__setitem__("h", h)
    m.get_axon_ntff_profile_hook = lambda: holder["h"]
    sys.modules["antenv.axon_hooks"] = m
    antenv.axon_hooks = m
    try:
        from trn_agent_boot.trn_boot import _ntff_profile_via_ctypes
        so = "/opt/axon/libaxon_pjrt.so"
        if os.path.exists(so):
            m.set_axon_ntff_profile_hook(_ntff_profile_via_ctypes(so))
    except Exception:
        pass


_ensure_axon_hooks()

import concourse.bass as bass
import concourse.tile as tile
from concourse import bacc, mybir
from concourse.bass_utils import run_bass_kernel_spmd
from concourse.masks import make_identity

F32 = mybir.dt.float32
F32R = mybir.dt.float32r
AF = mybir.ActivationFunctionType
ALU = mybir.AluOpType

B, D, N = 4096, 512, 5994
NCORES = 8
P = 128
BSH = B // NCORES          # 512 rows of x per core
BT = BSH // P              # 4 b-tiles
DT = D // P                # 4 d-tiles (contraction)
NSL = 750                  # per-core W n-slice width for the norm partials
CHUNK = 512                # output n-chunk (one PSUM bank of fp32)
EPS = 1e-12

NPASS = int(os.environ.get("COSSIM_NPASS", "3"))


def _chunks():
    res, n0 = [], 0
    while n0 < N:
        res.append((n0, min(CHUNK, N - n0)))
        n0 += CHUNK
    return res


def _build(npass: int):
    nc = bacc.Bacc("TRN2", target_bir_lowering=False, debug=False,
                   num_devices=NCORES)

    x_d = nc.dram_tensor("x", [BSH, D], F32, kind="ExternalInput").ap()
    w_d = nc.dram_tensor("W", [D, N], F32, kind="ExternalInput").ap()
    wsl_d = nc.dram_tensor("wslice", [D, NSL], F32, kind="ExternalInput").ap()
    o_d = nc.dram_tensor("out", [BSH, N], F32, kind="ExternalOutput").ap()

    x_r = x_d.rearrange("(t p) d -> p t d", p=P)        # [128, 4, 512]
    w_r = w_d.rearrange("(t p) n -> p t n", p=P)        # [128, 4, 5994]
    wsl_r = wsl_d.rearrange("(t p) n -> p t n", p=P)    # [128, 4, 750]
    o_r = o_d.rearrange("(t p) n -> p t n", p=P)        # [128, 4, 5994]

    with tile.TileContext(nc) as tc, ExitStack() as ctx:
        const = ctx.enter_context(tc.tile_pool(name="const", bufs=1))
        xp = ctx.enter_context(tc.tile_pool(name="xp", bufs=1))
        sq = ctx.enter_context(tc.tile_pool(name="sq", bufs=1))
        sc = ctx.enter_context(tc.tile_pool(name="sc", bufs=1))
        xt = ctx.enter_context(tc.tile_pool(name="xt", bufs=1))
        wfp = ctx.enter_context(tc.tile_pool(name="wfp", bufs=3))
        wrp = ctx.enter_context(tc.tile_pool(name="wrp", bufs=3))
        ostp = ctx.enter_context(tc.tile_pool(name="ostp", bufs=3))
        dram = ctx.enter_context(tc.tile_pool(name="dram", bufs=1, space="DRAM"))
        tp = ctx.enter_context(tc.tile_pool(name="tp", bufs=2, space="PSUM"))
        mm = ctx.enter_context(tc.tile_pool(name="mm", bufs=4, space="PSUM"))

        identity = const.tile([P, P], F32)
        make_identity(nc, identity)

        # ---- x: load, row sum-of-squares -> xscale ----
        x_sb = xp.tile([P, BT, D], F32)
        nc.sync.dma_start(x_sb, x_r)
        xsq = sc.tile([P, BT], F32)
        for bt in range(BT):
            trash = sq.tile([P, D], F32, tag="trx")
            nc.scalar.activation(trash, x_sb[:, bt, :], AF.Square,
                                 accum_out=xsq[:, bt:bt + 1])
        xmx = sc.tile([P, BT], F32)
        nc.vector.tensor_scalar_max(xmx, xsq, EPS)
        xsr = sc.tile([P, BT], F32)
        nc.scalar.sqrt(xsr, xmx)
        xsc = sc.tile([P, BT], F32)
        nc.vector.reciprocal(xsc, xsr)

        # ---- W row-norm partials over this core's slice + AllReduce ----
        wsl_sb = sq.tile([P, DT, NSL], F32, tag="wsl")
        nc.sync.dma_start(wsl_sb, wsl_r)
        wsqp = sc.tile([P, DT], F32)
        for dt in range(DT):
            trashw = sq.tile([P, NSL], F32, tag="trw")
            nc.scalar.activation(trashw, wsl_sb[:, dt, :], AF.Square,
                                 accum_out=wsqp[:, dt:dt + 1])
        cc_in = dram.tile([P, DT], F32)
        cc_out = dram.tile([P, DT], F32)
        nc.gpsimd.dma_start(cc_in[:], wsqp)
        nc.gpsimd.collective_compute(
            "AllReduce", ALU.add,
            replica_groups=[list(range(NCORES))],
            ins=[cc_in.opt()], outs=[cc_out.opt()],
        )
        wsq = sc.tile([P, DT], F32)
        nc.gpsimd.dma_start(wsq, cc_out[:])
        wmx = sc.tile([P, DT], F32)
        nc.vector.tensor_scalar_max(wmx, wsq, EPS)
        wsr = sc.tile([P, DT], F32)
        nc.scalar.sqrt(wsr, wmx)
        wsc = sc.tile([P, DT], F32)
        nc.vector.reciprocal(wsc, wsr)

        # ---- transpose x, fold in wscale, split to f32r (+ residual) ----
        xtr1 = xt.tile([P, DT, BSH], F32R, tag="xtr1")
        xtr2 = None
        if npass >= 2:
            xtr2 = xt.tile([P, DT, BSH], F32R, tag="xtr2")
        for dt in range(DT):
            for bt in range(BT):
                pt = tp.tile([P, P], F32)
                nc.tensor.transpose(pt, x_sb[:, bt, dt * P:(dt + 1) * P],
                                    identity)
                dst1 = xtr1[:, dt, bt * P:(bt + 1) * P]
                nc.scalar.activation(dst1, pt, AF.Copy,
                                     scale=wsc[:, dt:dt + 1])
                if npass >= 2:
                    nc.vector.scalar_tensor_tensor(
                        out=xtr2[:, dt, bt * P:(bt + 1) * P],
                        in0=pt, scalar=wsc[:, dt:dt + 1], in1=dst1,
                        op0=ALU.mult, op1=ALU.subtract)

        # ---- stream W chunks: round to f32r (+ residual), matmul, evict ----
        for n0, nw in _chunks():
            wf = wfp.tile([P, DT, CHUNK], F32, tag="wf")
            nc.sync.dma_start(wf[:, :, :nw], w_r[:, :, n0:n0 + nw])
            wr1 = wrp.tile([P, DT, CHUNK], F32R, tag="wr1")
            for dt in range(DT):
                nc.vector.tensor_copy(wr1[:, dt, :nw], wf[:, dt, :nw])
            wr2 = None
            if npass >= 3:
                wr2 = wrp.tile([P, DT, CHUNK], F32R, tag="wr2")
                for dt in range(DT):
                    nc.vector.scalar_tensor_tensor(
                        out=wr2[:, dt, :nw], in0=wf[:, dt, :nw], scalar=1.0,
                        in1=wr1[:, dt, :nw], op0=ALU.mult, op1=ALU.subtract)

            terms = [(xtr1, wr1)]
            if npass >= 2:
                terms.append((xtr2, wr1))
            if npass >= 3:
                terms.append((xtr1, wr2))

            ost = ostp.tile([P, BT, CHUNK], F32, tag="ost")
            nmm = len(terms) * DT
            for bt in range(BT):
                ps = mm.tile([P, CHUNK], F32)
                i = 0
                for xs, ws in terms:
                    for dt in range(DT):
                        nc.tensor.matmul(
                            ps[:, :nw],
                            xs[:, dt, bt * P:(bt + 1) * P],
                            ws[:, dt, :nw],
                            start=(i == 0), stop=(i == nmm - 1))
                        i += 1
                nc.scalar.activation(ost[:, bt, :nw], ps[:, :nw], AF.Copy,
                                     scale=xsc[:, bt:bt + 1])
            nc.sync.dma_start(o_r[:, :, n0:n0 + nw], ost[:, :, :nw])

    nc.compile()
    return nc


LAST_RESULT = None


def kernel(x: np.ndarray, W: np.ndarray) -> np.ndarray:
    global LAST_RESULT
    x = np.ascontiguousarray(x, dtype=np.float32)
    W = np.ascontiguousarray(W, dtype=np.float32)
    assert x.shape == (B, D) and W.shape == (D, N)

    nc = _build(NPASS)

    in_maps = []
    for c in range(NCORES):
        s = c * NSL
        wslc = np.zeros((D, NSL), dtype=np.float32)
        width = max(0, min(NSL, N - s))
        if width:
            wslc[:, :width] = W[:, s:s + width]
        in_maps.append({
            "x": np.ascontiguousarray(x[c * BSH:(c + 1) * BSH]),
            "W": W,
            "wslice": wslc,
        })

    res = run_bass_kernel_spmd(nc, in_maps, core_ids=list(range(NCORES)))
    LAST_RESULT = res
    return np.concatenate([res.results[c]["out"] for c in range(NCORES)],
                          axis=0)


# revision 6
# speedup vs baseline: 1.3671x; 1.3671x over previous
"""Trainium2 Bass kernel for nn_CosineSimilarityLayer.

out = l2norm_rows(x) @ (l2norm_rows_over_N(W))            x:[4096,512] W:[512,5994]

Math:  out[b,n] = xscale[b] * sum_d x[b,d] * wscale[d] * W[d,n]
  xscale[b] = rsqrt(max(sum_d x[b,d]^2, eps))   (folded into PSUM eviction)
  wscale[d] = rsqrt(max(sum_n W[d,n]^2, eps))   (folded into transposed x)

Sharding: data-parallel over batch, 8 cores x [512, 512] shards, W replicated.
The W row-norm (a full-N reduction) is computed cooperatively: each core
squares+reduces a 750-col slice of W, then an 8-core AllReduce produces the
full sums, so matmuls start ~15us in instead of waiting for a full W pass.

Matmul runs in float32r (fast fp32 mode, ~13-bit mantissa per pass).  NPASS
controls precision: each operand is split into float32r value + float32r
residual; 3 passes (hi*hi + lo*hi + hi*lo) recovers ~fp32 accuracy.
"""

import os
import sys
import types
from contextlib import ExitStack

import numpy as np


def _ensure_axon_hooks():
    """bass_utils' trace path imports antenv.axon_hooks, which some images
    lack.  Provide it (wired to the ctypes NTFF hook when available) so
    BASS_TRACE=1 profiles instead of crashing.  No-op when already present."""
    try:
        import antenv.axon_hooks  # noqa: F401
        return
    except ImportError:
        pass
    try:
        import antenv
    except ImportError:
        return
    m = types.ModuleType("antenv.axon_hooks")
    holder = {"h": None}
    m.set_axon_ntff_profile_hook = lambda h: holder.__setitem__("h", h)
    m.get_axon_ntff_profile_hook = lambda: holder["h"]
    sys.modules["antenv.axon_hooks"] = m
    antenv.axon_hooks = m
    try:
        from trn_agent_boot.trn_boot import _ntff_profile_via_ctypes
        so = "/opt/axon/libaxon_pjrt.so"
        if os.path.exists(so):
            m.set_axon_ntff_profile_hook(_ntff_profile_via_ctypes(so))
    except Exception:
        pass


_ensure_axon_hooks()

import concourse.bass as bass
import concourse.tile as tile
from concourse import bacc, mybir
from concourse.bass_utils import run_bass_kernel_spmd
from concourse.masks import make_identity

F32 = mybir.dt.float32
F32R = mybir.dt.float32r
AF = mybir.ActivationFunctionType
ALU = mybir.AluOpType

B, D, N = 4096, 512, 5994
NCORES = 8
P = 128
BSH = B // NCORES          # 512 rows of x per core
BT = BSH // P              # 4 b-tiles
DT = D // P                # 4 d-tiles (contraction)
NSL = 750                  # per-core W n-slice width for the norm partials
CHUNK = 512                # output n-chunk (one PSUM bank of fp32)
EPS = 1e-12

NPASS = int(os.environ.get("COSSIM_NPASS", "3"))


def _chunks():
    res, n0 = [], 0
    while n0 < N:
        res.append((n0, min(CHUNK, N - n0)))
        n0 += CHUNK
    return res


def _build(npass: int):
    nc = bacc.Bacc("TRN2", target_bir_lowering=False, debug=False,
                   num_devices=NCORES)

    x_d = nc.dram_tensor("x", [BSH, D], F32, kind="ExternalInput").ap()
    w_d = nc.dram_tensor("W", [D, N], F32, kind="ExternalInput").ap()
    wsl_d = nc.dram_tensor("wslice", [D, NSL], F32, kind="ExternalInput").ap()
    o_d = nc.dram_tensor("out", [BSH, N], F32, kind="ExternalOutput").ap()

    x_r = x_d.rearrange("(t p) d -> p t d", p=P)        # [128, 4, 512]
    w_r = w_d.rearrange("(t p) n -> p t n", p=P)        # [128, 4, 5994]
    wsl_r = wsl_d.rearrange("(t p) n -> p t n", p=P)    # [128, 4, 750]
    o_r = o_d.rearrange("(t p) n -> p t n", p=P)        # [128, 4, 5994]

    with tile.TileContext(nc) as tc, ExitStack() as ctx:
        const = ctx.enter_context(tc.tile_pool(name="const", bufs=1))
        xp = ctx.enter_context(tc.tile_pool(name="xp", bufs=1))
        sq = ctx.enter_context(tc.tile_pool(name="sq", bufs=1))
        sc = ctx.enter_context(tc.tile_pool(name="sc", bufs=1))
        xt = ctx.enter_context(tc.tile_pool(name="xt", bufs=1))
        wfp = ctx.enter_context(tc.tile_pool(name="wfp", bufs=3))
        wrp = ctx.enter_context(tc.tile_pool(name="wrp", bufs=3))
        ostp = ctx.enter_context(tc.tile_pool(name="ostp", bufs=3))
        dram = ctx.enter_context(tc.tile_pool(name="dram", bufs=1, space="DRAM"))
        tp = ctx.enter_context(tc.tile_pool(name="tp", bufs=2, space="PSUM"))
        mm = ctx.enter_context(tc.tile_pool(name="mm", bufs=4, space="PSUM"))

        # ---- W row-norm partials over this core's slice + AllReduce ----
        # Emitted FIRST: the AllReduce's small DMA hops crawl when the big
        # W-streaming traffic saturates the SDMA engines (measured 60us for a
        # 2KB allreduce under load), so the heavy DMAs are gated on it below.
        wsl_sb = sq.tile([P, DT, NSL], F32, tag="wsl")
        nc.sync.dma_start(wsl_sb, wsl_r)
        wsqp = sc.tile([P, DT], F32)
        for dt in range(DT):
            trashw = sq.tile([P, NSL], F32, tag="trw")
            nc.scalar.activation(trashw, wsl_sb[:, dt, :], AF.Square,
                                 accum_out=wsqp[:, dt:dt + 1])
        cc_in = dram.tile([P, DT], F32)
        cc_out = dram.tile([P, DT], F32)
        nc.gpsimd.dma_start(cc_in[:], wsqp)
        cc_inst = nc.gpsimd.collective_compute(
            "AllReduce", ALU.add,
            replica_groups=[list(range(NCORES))],
            ins=[cc_in.opt()], outs=[cc_out.opt()],
        )
        wsq = sc.tile([P, DT], F32)
        nc.gpsimd.dma_start(wsq, cc_out[:])
        wmx = sc.tile([P, DT], F32)
        nc.vector.tensor_scalar_max(wmx, wsq, EPS)
        wsr = sc.tile([P, DT], F32)
        nc.scalar.sqrt(wsr, wmx)
        wsc = sc.tile([P, DT], F32)
        nc.vector.reciprocal(wsc, wsr)

        # ---- x: load, row sum-of-squares -> xscale ----
        x_sb = xp.tile([P, BT, D], F32)
        nc.sync.dma_start(x_sb, x_r)
        xsq = sc.tile([P, BT], F32)
        for bt in range(BT):
            trash = sq.tile([P, D], F32, tag="trx")
            nc.scalar.activation(trash, x_sb[:, bt, :], AF.Square,
                                 accum_out=xsq[:, bt:bt + 1])
        xmx = sc.tile([P, BT], F32)
        nc.vector.tensor_scalar_max(xmx, xsq, EPS)
        xsr = sc.tile([P, BT], F32)
        nc.scalar.sqrt(xsr, xmx)
        xsc = sc.tile([P, BT], F32)
        nc.vector.reciprocal(xsc, xsr)

        identity = const.tile([P, P], F32)
        make_identity(nc, identity)

        # ---- transpose x, fold in wscale, split to f32r (+ residual) ----
        xtr1 = xt.tile([P, DT, BSH], F32R, tag="xtr1")
        xtr2 = None
        if npass >= 2:
            xtr2 = xt.tile([P, DT, BSH], F32R, tag="xtr2")
        for dt in range(DT):
            for bt in range(BT):
                pt = tp.tile([P, P], F32)
                nc.tensor.transpose(pt, x_sb[:, bt, dt * P:(dt + 1) * P],
                                    identity)
                dst1 = xtr1[:, dt, bt * P:(bt + 1) * P]
                nc.scalar.activation(dst1, pt, AF.Copy,
                                     scale=wsc[:, dt:dt + 1])
                if npass >= 2:
                    nc.vector.scalar_tensor_tensor(
                        out=xtr2[:, dt, bt * P:(bt + 1) * P],
                        in0=pt, scalar=wsc[:, dt:dt + 1], in1=dst1,
                        op0=ALU.mult, op1=ALU.subtract)

        # ---- stream W chunks: round to f32r (+ residual), matmul, evict ----
        for n0, nw in _chunks():
            wf = wfp.tile([P, DT, CHUNK], F32, tag="wf")
            wdma = nc.sync.dma_start(wf[:, :, :nw], w_r[:, :, n0:n0 + nw])
            # Hold the heavy W stream until the AllReduce is done so the
            # collective's hops don't contend with it on the SDMA engines.
            tile.add_dep_helper(wdma.ins, cc_inst.ins, sync=True,
                                reason="W stream after allreduce")
            wr1 = wrp.tile([P, DT, CHUNK], F32R, tag="wr1")
            for dt in range(DT):
                nc.vector.tensor_copy(wr1[:, dt, :nw], wf[:, dt, :nw])
            wr2 = None
            if npass >= 3:
                wr2 = wrp.tile([P, DT, CHUNK], F32R, tag="wr2")
                for dt in range(DT):
                    nc.vector.scalar_tensor_tensor(
                        out=wr2[:, dt, :nw], in0=wf[:, dt, :nw], scalar=1.0,
                        in1=wr1[:, dt, :nw], op0=ALU.mult, op1=ALU.subtract)

            terms = [(xtr1, wr1)]
            if npass >= 2:
                terms.append((xtr2, wr1))
            if npass >= 3:
                terms.append((xtr1, wr2))

            ost = ostp.tile([P, BT, CHUNK], F32, tag="ost")
            nmm = len(terms) * DT
            for bt in range(BT):
                ps = mm.tile([P, CHUNK], F32)
                i = 0
                for xs, ws in terms:
                    for dt in range(DT):
                        nc.tensor.matmul(
                            ps[:, :nw],
                            xs[:, dt, bt * P:(bt + 1) * P],
                            ws[:, dt, :nw],
                            start=(i == 0), stop=(i == nmm - 1))
                        i += 1
                nc.scalar.activation(ost[:, bt, :nw], ps[:, :nw], AF.Copy,
                                     scale=xsc[:, bt:bt + 1])
            # Output DMA on the Activation HWDGE ring so it never blocks the
            # sync sequencer that issues the W-in stream.
            nc.scalar.dma_start(o_r[:, :, n0:n0 + nw], ost[:, :, :nw])

    nc.compile()
    return nc


LAST_RESULT = None


def kernel(x: np.ndarray, W: np.ndarray) -> np.ndarray:
    global LAST_RESULT
    x = np.ascontiguousarray(x, dtype=np.float32)
    W = np.ascontiguousarray(W, dtype=np.float32)
    assert x.shape == (B, D) and W.shape == (D, N)

    nc = _build(NPASS)

    in_maps = []
    for c in range(NCORES):
        s = c * NSL
        wslc = np.zeros((D, NSL), dtype=np.float32)
        width = max(0, min(NSL, N - s))
        if width:
            wslc[:, :width] = W[:, s:s + width]
        in_maps.append({
            "x": np.ascontiguousarray(x[c * BSH:(c + 1) * BSH]),
            "W": W,
            "wslice": wslc,
        })

    res = run_bass_kernel_spmd(nc, in_maps, core_ids=list(range(NCORES)))
    LAST_RESULT = res
    return np.concatenate([res.results[c]["out"] for c in range(NCORES)],
                          axis=0)


# revision 7
# speedup vs baseline: 2.1227x; 1.5527x over previous
"""Trainium2 Bass kernel for nn_CosineSimilarityLayer.

out = l2norm_rows(x) @ l2norm_rows_over_N(W)       x:[4096,512]  W:[512,5994]

Math:  out[b,n] = xscale[b] * sum_d x[b,d] * wscale[d] * W[d,n]
  xscale[b] = rsqrt(max(sum_d x[b,d]^2, eps))   (folded into PSUM eviction)
  wscale[d] = rsqrt(max(sum_n W[d,n]^2, eps))   (folded into transposed x)

Sharding: data-parallel over batch — 8 cores x [512, 512] x-shards, W
replicated.  Everything is core-local (no collectives): a cross-core sync
point would bill every core for the slowest core's NEFF start (tens of us of
input-upload skew), which costs more than the redundant W-norm work it saves.

Per core: W streams in once as 512-column chunks and stays resident in SBUF;
each chunk is squared+row-accumulated on the Scalar engine as it lands, so
wscale is ready right after the last chunk.  x is PE-transposed (via
identity) during the W stream; once wscale lands, xT is scaled+rounded to
float32r and the chunk loop runs 4 accumulating matmuls per output tile
(x NPASS for split-precision), evicting PSUM through the Scalar engine with
the per-row xscale, with output DMA on the Activation HWDGE ring.

float32r is the fast fp32 matmul mode (~13-bit effective mantissa, full
speed for free dim >= 256).  NPASS=1 (default): rel err ~1.6e-4.  NPASS=3
splits each operand into value + residual (hi*hi + lo*hi + hi*lo) and
recovers ~fp32 accuracy (~2e-6) at 3x the PE time.
"""

import os
import sys
import types
from contextlib import ExitStack

import numpy as np


def _ensure_axon_hooks():
    """bass_utils' trace path imports antenv.axon_hooks, which some images
    lack.  Provide it (wired to the ctypes NTFF hook when available) so
    BASS_TRACE=1 profiles instead of crashing.  No-op when already present."""
    try:
        import antenv.axon_hooks  # noqa: F401
        return
    except ImportError:
        pass
    try:
        import antenv
    except ImportError:
        return
    m = types.ModuleType("antenv.axon_hooks")
    holder = {"h": None}
    m.set_axon_ntff_profile_hook = lambda h: holder.__setitem__("h", h)
    m.get_axon_ntff_profile_hook = lambda: holder["h"]
    sys.modules["antenv.axon_hooks"] = m
    antenv.axon_hooks = m
    try:
        from trn_agent_boot.trn_boot import _ntff_profile_via_ctypes
        so = "/opt/axon/libaxon_pjrt.so"
        if os.path.exists(so):
            m.set_axon_ntff_profile_hook(_ntff_profile_via_ctypes(so))
    except Exception:
        pass


_ensure_axon_hooks()

import concourse.bass as bass
import concourse.tile as tile
from concourse import bacc, mybir
from concourse.bass_utils import run_bass_kernel_spmd
from concourse.masks import make_identity

F32 = mybir.dt.float32
F32R = mybir.dt.float32r
AF = mybir.ActivationFunctionType
ALU = mybir.AluOpType

B, D, N = 4096, 512, 5994
NCORES = 8
P = 128
BSH = B // NCORES          # 512 rows of x per core
BT = BSH // P              # 4 b-tiles
DT = D // P                # 4 d-tiles (contraction)
CHUNK = 512                # output n-chunk (one PSUM bank of fp32)
EPS = 1e-12

NPASS = int(os.environ.get("COSSIM_NPASS", "1"))

CHUNKS = []
_n0 = 0
while _n0 < N:
    CHUNKS.append((_n0, min(CHUNK, N - _n0)))
    _n0 += CHUNK
NCH = len(CHUNKS)          # 12


def _build(npass: int):
    nc = bacc.Bacc("TRN2", target_bir_lowering=False, debug=False,
                   num_devices=NCORES)

    x_d = nc.dram_tensor("x", [BSH, D], F32, kind="ExternalInput").ap()
    w_d = nc.dram_tensor("W", [D, N], F32, kind="ExternalInput").ap()
    o_d = nc.dram_tensor("out", [BSH, N], F32, kind="ExternalOutput").ap()

    x_r = x_d.rearrange("(t p) d -> p t d", p=P)        # [128, 4, 512]
    w_r = w_d.rearrange("(t p) n -> p t n", p=P)        # [128, 4, 5994]
    o_r = o_d.rearrange("(t p) n -> p t n", p=P)        # [128, 4, 5994]

    with tile.TileContext(nc) as tc, ExitStack() as ctx:
        const = ctx.enter_context(tc.tile_pool(name="const", bufs=1))
        xp = ctx.enter_context(tc.tile_pool(name="xp", bufs=1))
        sq = ctx.enter_context(tc.tile_pool(name="sq", bufs=2))
        sc = ctx.enter_context(tc.tile_pool(name="sc", bufs=1))
        xt = ctx.enter_context(tc.tile_pool(name="xt", bufs=1))
        wsb = ctx.enter_context(tc.tile_pool(name="wsb", bufs=1))
        wrp = ctx.enter_context(tc.tile_pool(name="wrp", bufs=3))
        ostp = ctx.enter_context(tc.tile_pool(name="ostp", bufs=3))
        tp = ctx.enter_context(tc.tile_pool(name="tp", bufs=2, space="PSUM"))
        mm = ctx.enter_context(tc.tile_pool(name="mm", bufs=4, space="PSUM"))

        # ---- W streams in once, stays resident; squares accumulate per
        # chunk so wscale is ready right after the last chunk lands ----
        w_sb = wsb.tile([P, DT, N], F32)
        wsqp = sc.tile([P, DT, NCH], F32)
        for ci, (n0, nw) in enumerate(CHUNKS):
            nc.sync.dma_start(w_sb[:, :, n0:n0 + nw], w_r[:, :, n0:n0 + nw])
            for dt in range(DT):
                trashw = sq.tile([P, CHUNK], F32, tag="trw")
                nc.scalar.activation(trashw[:, :nw], w_sb[:, dt, n0:n0 + nw],
                                     AF.Square,
                                     accum_out=wsqp[:, dt, ci:ci + 1])
        wsq = sc.tile([P, DT, 1], F32)
        nc.vector.reduce_sum(wsq, wsqp, axis=mybir.AxisListType.X)
        wmx = sc.tile([P, DT, 1], F32)
        nc.vector.tensor_scalar_max(wmx, wsq, EPS)
        wsr = sc.tile([P, DT, 1], F32)
        nc.scalar.sqrt(wsr, wmx)
        wsc = sc.tile([P, DT, 1], F32)
        nc.vector.reciprocal(wsc, wsr)

        # ---- x: load, row sum-of-squares -> xscale ----
        x_sb = xp.tile([P, BT, D], F32)
        nc.sync.dma_start(x_sb, x_r)
        xsq = sc.tile([P, BT], F32)
        for bt in range(BT):
            trash = sq.tile([P, D], F32, tag="trx")
            nc.scalar.activation(trash, x_sb[:, bt, :], AF.Square,
                                 accum_out=xsq[:, bt:bt + 1])
        xmx = sc.tile([P, BT], F32)
        nc.vector.tensor_scalar_max(xmx, xsq, EPS)
        xsr = sc.tile([P, BT], F32)
        nc.scalar.sqrt(xsr, xmx)
        xsc = sc.tile([P, BT], F32)
        nc.vector.reciprocal(xsc, xsr)

        identity = const.tile([P, P], F32)
        make_identity(nc, identity)

        # ---- transpose x early (overlaps the W stream), then scale+round
        # to f32r once wscale is ready ----
        xtf = xt.tile([P, DT, BSH], F32, tag="xtf")
        for dt in range(DT):
            for bt in range(BT):
                pt = tp.tile([P, P], F32)
                nc.tensor.transpose(pt, x_sb[:, bt, dt * P:(dt + 1) * P],
                                    identity)
                nc.vector.tensor_copy(xtf[:, dt, bt * P:(bt + 1) * P], pt)
        xtr1 = xt.tile([P, DT, BSH], F32R, tag="xtr1")
        xtr2 = None
        if npass >= 2:
            xtr2 = xt.tile([P, DT, BSH], F32R, tag="xtr2")
        for dt in range(DT):
            nc.scalar.activation(xtr1[:, dt, :], xtf[:, dt, :], AF.Copy,
                                 scale=wsc[:, dt, :])
            if npass >= 2:
                nc.vector.scalar_tensor_tensor(
                    out=xtr2[:, dt, :], in0=xtf[:, dt, :],
                    scalar=wsc[:, dt, :], in1=xtr1[:, dt, :],
                    op0=ALU.mult, op1=ALU.subtract)

        # ---- chunk loop: round W slice to f32r, matmul, evict, store ----
        for n0, nw in CHUNKS:
            wr1 = wrp.tile([P, DT, CHUNK], F32R, tag="wr1")
            for dt in range(DT):
                nc.vector.tensor_copy(wr1[:, dt, :nw], w_sb[:, dt, n0:n0 + nw])
            wr2 = None
            if npass >= 3:
                wr2 = wrp.tile([P, DT, CHUNK], F32R, tag="wr2")
                for dt in range(DT):
                    nc.vector.scalar_tensor_tensor(
                        out=wr2[:, dt, :nw], in0=w_sb[:, dt, n0:n0 + nw],
                        scalar=1.0, in1=wr1[:, dt, :nw],
                        op0=ALU.mult, op1=ALU.subtract)

            terms = [(xtr1, wr1)]
            if npass >= 2:
                terms.append((xtr2, wr1))
            if npass >= 3:
                terms.append((xtr1, wr2))

            ost = ostp.tile([P, BT, CHUNK], F32, tag="ost")
            nmm = len(terms) * DT
            for bt in range(BT):
                ps = mm.tile([P, CHUNK], F32)
                i = 0
                for xs, ws in terms:
                    for dt in range(DT):
                        nc.tensor.matmul(
                            ps[:, :nw],
                            xs[:, dt, bt * P:(bt + 1) * P],
                            ws[:, dt, :nw],
                            start=(i == 0), stop=(i == nmm - 1))
                        i += 1
                nc.scalar.activation(ost[:, bt, :nw], ps[:, :nw], AF.Copy,
                                     scale=xsc[:, bt:bt + 1])
            # Output DMA on the Activation HWDGE ring so it never blocks the
            # sync sequencer that issues the W-in stream.
            nc.scalar.dma_start(o_r[:, :, n0:n0 + nw], ost[:, :, :nw])

    nc.compile()
    return nc


LAST_RESULT = None


def kernel(x: np.ndarray, W: np.ndarray) -> np.ndarray:
    global LAST_RESULT
    x = np.ascontiguousarray(x, dtype=np.float32)
    W = np.ascontiguousarray(W, dtype=np.float32)
    assert x.shape == (B, D) and W.shape == (D, N)

    nc = _build(NPASS)

    in_maps = [{"x": np.ascontiguousarray(x[c * BSH:(c + 1) * BSH]), "W": W}
               for c in range(NCORES)]

    res = run_bass_kernel_spmd(nc, in_maps, core_ids=list(range(NCORES)))
    LAST_RESULT = res
    return np.concatenate([res.results[c]["out"] for c in range(NCORES)],
                          axis=0)
